# revision 2
# baseline (speedup 1.0000x reference)
"""CapsNet dynamic-routing layer on 8 Trainium2 NeuronCores.

Strategy
--------
Shard the R=512 routes across 8 cores (64 each); W is read exactly once
machine-wide. Per core:

  u_hat[b, r, c, o] = sum_i W[r,c,o,i] * x[b,r,i]
    via TensorE: stationary = x[r] as [I=128, B=32] fp16 hi/lo pairs,
    moving = W[r] as [I=128, co-chunk 512] fp16 hi/lo pairs, 3 passes
    (hh + lh + hl) accumulated in PSUM -> ~fp32 precision at bf16 speed.
    4 routes run concurrently via tile_position col-strips; PSUM bank
    [128=(rj, b), 512] evacuated to SBUF u_hat [128, g, co].

  Routing iteration 0's s = (1/C) sum_r u_hat comes free on TensorE:
    block-diag delta/32 stationary matmuls accumulate over r in PSUM.

  Iterations 1-2: c_ij mult + segmented reductions on VectorE; the
  cross-partition (rj) sum of s via fp32 delta matmul; softmax/exp/sqrt
  on ScalarE; s AllReduced across cores through DRAM (256 KB).

  Output v = squash(s) computed in a c-packed [128, 512] layout (8x
  cheaper reciprocal), broadcast back through DRAM for the agreement
  passes. All cores produce identical v; core 0's copy is returned.

Numerics: fp16 hi+lo splits carry ~22 mantissa bits; measured end-to-end
error matches pure-fp32 arithmetic (~1.3e-4 on v, routing amplifies any
u_hat error ~1000x, which rules out bf16/tf32 anywhere on the hot path).
"""
import sys

sys.path.insert(0, "/opt/trn_rl_repo")

import numpy as np

import concourse.bass as bass
import concourse.tile as tile
from concourse import mybir
from concourse.bass_utils import run_bass_kernel_spmd

F16 = mybir.dt.float16
F32 = mybir.dt.float32

NCORES = 8
B, R, C, O, I = 32, 512, 32, 64, 128
CO = C * O                # 2048
RL = R // NCORES          # 64 routes per core
J = 4                     # col-strips (rj)
G = RL // J               # 16 r-groups
NQ = 4                    # co chunks
Q = CO // NQ              # 512
C8 = C // 4               # free-c in packed layout
EPS = 1e-8

_cache = {}


def _legalize_install(nc):
    """This walrus build accepts at most one sync wait per instruction and
    none on Matmult; hoist extras onto standalone EventSemaphore ops."""
    import json
    from concourse import mybir as _mb

    def legalize(raw: bytes) -> bytes:
        d = json.loads(raw)
        ctr = 0
        for f in d.get("functions", []):
            for blk in f.get("blocks", []):
                out = []
                for ins in blk.get("instructions", []):
                    si = ins.get("sync_info")
                    waits = (si or {}).get("on_wait") or []
                    keep = 0 if ins.get("opcode") in ("Matmult", "Ldweights") else 1
                    if len(waits) > keep:
                        nh = len(waits) - keep
                        for w in waits[:nh]:
                            ctr += 1
                            out.append({
                                "debug": ins.get("debug", 0),
                                "engine": ins["engine"],
                                "ins": [], "outs": [],
                                "name": f"lgl_wait_{ctr}",
                                "opcode": "EventSemaphore",
                                "sync_info": {"on_update": [], "on_wait": [w]},
                            })
                        si["on_wait"] = waits[nh:]
                    out.append(ins)
                blk["instructions"] = out
        return json.dumps(d).encode()

    nc.to_json_bytes = lambda: legalize(_mb.module_to_json_bytes(nc.m))
    return nc


def _build():
    nc = bass.Bass(trn_type="TRN2", target_bir_lowering=False, debug=False,
                   num_devices=NCORES)

    d_xh = nc.dram_tensor("xh", [I, RL, B], F16, kind="ExternalInput").ap()
    d_xl = nc.dram_tensor("xl", [I, RL, B], F16, kind="ExternalInput").ap()
    d_Wh = nc.dram_tensor("Wh", [RL, I, CO], F16, kind="ExternalInput").ap()
    d_Wl = nc.dram_tensor("Wl", [RL, I, CO], F16, kind="ExternalInput").ap()
    d_d0 = nc.dram_tensor("delta_s0", [128, B], F32, kind="ExternalInput").ap()
    d_d1 = nc.dram_tensor("delta_1", [128, B], F32, kind="ExternalInput").ap()
    d_vout = nc.dram_tensor("v_out", [B, CO], F32, kind="ExternalOutput").ap()

    d_sb = [nc.dram_tensor(f"s_bounce{t}", [B, CO], F32).ap() for t in range(3)]
    d_sr = [nc.dram_tensor(f"s_red{t}", [B, CO], F32, addr_space="Shared").ap()
            for t in range(3)]
    d_vdr = [nc.dram_tensor(f"v_dr{t}", [B, CO], F32).ap() for t in range(2)]

    groups = [list(range(NCORES))]

    with tile.TileContext(nc) as tc:
        with tc.tile_pool(name="const", bufs=1) as cpool, \
             tc.tile_pool(name="upool", bufs=1) as upool, \
             tc.tile_pool(name="s0ps", bufs=1, space="PSUM") as s0ps:

            t_xh = cpool.tile([I, RL * B], F16, tag="xh")
            t_xl = cpool.tile([I, RL * B], F16, tag="xl")
            nc.sync.dma_start(t_xh[:].rearrange("i (r b) -> i r b", r=RL), d_xh)
            nc.sync.dma_start(t_xl[:].rearrange("i (r b) -> i r b", r=RL), d_xl)
            t_d0 = cpool.tile([128, B], F32, tag="d0")
            t_d1 = cpool.tile([128, B], F32, tag="d1")
            nc.sync.dma_start(t_d0[:], d_d0)
            nc.sync.dma_start(t_d1[:], d_d1)
            t_eps = cpool.tile([128, 1], F32, tag="eps")
            nc.gpsimd.memset(t_eps[:], EPS)

            t_u = upool.tile([128, G, CO], F32, tag="u")
            s0q = [s0ps.tile([B, Q], F32, tag=f"s0q{q}", name=f"s0q{q}")
                   for q in range(NQ)]

            # ---- Phase A: u_hat production + iter-0 s accumulation ----
            with tc.tile_pool(name="wpool", bufs=3) as wpool, \
                 tc.tile_pool(name="prodps", bufs=2, space="PSUM") as prodps:
                pend = None  # (g, q) whose s0-matmul is deferred one step
                for g in range(G):
                    for q in range(NQ):
                        wh = wpool.tile([I, J, Q], F16, tag="wh")
                        wl = wpool.tile([I, J, Q], F16, tag="wl")
                        nc.sync.dma_start(
                            wh[:], d_Wh[J * g:J * g + J, :, Q * q:Q * q + Q]
                            .transpose([1, 0, 2]))
                        nc.sync.dma_start(
                            wl[:], d_Wl[J * g:J * g + J, :, Q * q:Q * q + Q]
                            .transpose([1, 0, 2]))
                        pp = prodps.tile([128, Q], F32, tag="prod")
                        for j in range(J):
                            r = J * g + j
                            sxh = t_xh[:, r * B:(r + 1) * B]
                            sxl = t_xl[:, r * B:(r + 1) * B]
                            tp = (0, 32 * j)
                            ppj = pp[32 * j:32 * (j + 1), :]
                            nc.tensor.matmul(ppj, sxh, wh[:, j, :],
                                             start=True, stop=False,
                                             tile_position=tp)
                            nc.tensor.matmul(ppj, sxl, wh[:, j, :],
                                             start=False, stop=False,
                                             tile_position=tp)
                            nc.tensor.matmul(ppj, sxh, wl[:, j, :],
                                             start=False, stop=True,
                                             tile_position=tp)
                        useg = t_u[:, g, Q * q:Q * q + Q]
                        if (g + q) % 2 == 0:
                            nc.vector.tensor_copy(useg, pp[:])
                        else:
                            nc.scalar.copy(useg, pp[:])
                        # deferred-by-one s0 accumulation keeps PE dense
                        if pend is not None:
                            pg, pq = pend
                            nc.tensor.matmul(
                                s0q[pq][:], t_d0[:],
                                t_u[:, pg, Q * pq:Q * pq + Q],
                                start=(pg == 0), stop=(pg == G - 1))
                        pend = (g, q)
                pg, pq = pend
                nc.tensor.matmul(s0q[pq][:], t_d0[:],
                                 t_u[:, pg, Q * pq:Q * pq + Q],
                                 start=(pg == 0), stop=(pg == G - 1))

            # ---- Phase B: routing iterations ----
            with tc.tile_pool(name="iter", bufs=1) as ip, \
                 tc.tile_pool(name="tmp", bufs=1) as tp_pool, \
                 tc.tile_pool(name="sps", bufs=1, space="PSUM") as sps:

                t_vrep = ip.tile([128, CO], F32, tag="vrep")
                t_spart = ip.tile([128, C, O], F32, tag="spart")
                t_b = ip.tile([128, G, C], F32, tag="bij")
                t_a = ip.tile([128, G, C], F32, tag="aij")
                t_cij = ip.tile([128, G, C], F32, tag="cij")
                t_mx = ip.tile([128, G], F32, tag="mx")
                t_rs = ip.tile([128, G], F32, tag="rs")
                t_sbounce = ip.tile([B, CO], F32, tag="sbounce")
                t_spk = ip.tile([128, NQ * C8 * O // NQ], F32, tag="spk")
                t_sq = ip.tile([128, C8 * O], F32, tag="sqt")
                t_rt = ip.tile([128, C8 * O], F32, tag="rt")
                t_vpk = ip.tile([128, C8 * O], F32, tag="vpk")

                u4 = t_u[:].rearrange("p g (c o) -> p g c o", c=C)

                def allreduce(t):
                    nc.sync.dma_start(d_sb[t], t_sbounce[:])
                    nc.gpsimd.collective_compute(
                        "AllReduce", mybir.AluOpType.add,
                        replica_groups=groups,
                        ins=[d_sb[t].opt()], outs=[d_sr[t].opt()])

                def squash(t):
                    """d_sr[t] -> packed v in t_vpk; partitions (c4, b),
                    free (c8, o) with c = c8*4 + c4."""
                    sr4 = d_sr[t].rearrange("b (c8 c4 o) -> b c8 c4 o",
                                            c4=4, o=O)
                    for c4 in range(4):
                        nc.sync.dma_start(
                            t_spk[32 * c4:32 * (c4 + 1), :]
                            .rearrange("b (c8 o) -> b c8 o", c8=C8),
                            sr4[:, :, c4, :])
                    nc.scalar.square(t_sq[:], t_spk[:])
                    nc.scalar.activation(t_rt[:], t_sq[:],
                                         mybir.ActivationFunctionType.Sqrt,
                                         bias=t_eps[:])
                    # rt := (1+sq)*sqrt(sq+eps); vpk := sq*s; rt := 1/rt
                    nc.vector.tensor_scalar_add(t_vpk[:], t_sq[:], 1.0)
                    nc.vector.tensor_mul(t_rt[:], t_rt[:], t_vpk[:])
                    nc.vector.reciprocal(t_rt[:], t_rt[:])
                    nc.vector.tensor_mul(t_sq[:], t_sq[:], t_spk[:])
                    nc.vector.tensor_mul(t_vpk[:], t_sq[:], t_rt[:])

                def v_to(dst):
                    d4 = dst.rearrange("b (c8 c4 o) -> b c8 c4 o", c4=4, o=O)
                    for c4 in range(4):
                        nc.sync.dma_start(
                            d4[:, :, c4, :],
                            t_vpk[32 * c4:32 * (c4 + 1), :]
                            .rearrange("b (c8 o) -> b c8 o", c8=C8))

                def vrep_from(src):
                    sflat = src.rearrange("b co -> b co")
                    for j in range(J):
                        nc.sync.dma_start(t_vrep[32 * j:32 * (j + 1), :], sflat)

                def a_pass(first):
                    dst = t_b if first else t_a
                    vr4 = t_vrep[:].rearrange("p (c o) -> p c o", c=C)
                    for cc in range(8):
                        cs = slice(4 * cc, 4 * cc + 4)
                        tmp2 = tp_pool.tile([128, G, 4, O], F32, tag="tmp")
                        in1 = vr4[:, cs, :].unsqueeze(1) \
                            .broadcast_to([128, G, 4, O])
                        nc.vector.tensor_tensor(tmp2[:], u4[:, :, cs, :], in1,
                                                mybir.AluOpType.mult)
                        nc.vector.tensor_reduce(dst[:, :, cs], tmp2[:],
                                                axis=mybir.AxisListType.X,
                                                op=mybir.AluOpType.add)
                    if not first:
                        nc.vector.tensor_add(t_b[:], t_b[:], t_a[:])

                def softmax():
                    nc.vector.tensor_reduce(t_mx[:], t_b[:],
                                            axis=mybir.AxisListType.X,
                                            op=mybir.AluOpType.max)
                    mxb = t_mx[:].unsqueeze(2).broadcast_to([128, G, C])
                    nc.vector.tensor_sub(t_cij[:], t_b[:], mxb)
                    nc.scalar.activation(t_cij[:], t_cij[:],
                                         mybir.ActivationFunctionType.Exp)
                    nc.vector.tensor_reduce(t_rs[:], t_cij[:],
                                            axis=mybir.AxisListType.X,
                                            op=mybir.AluOpType.add)
                    nc.vector.reciprocal(t_rs[:], t_rs[:])
                    rsb = t_rs[:].unsqueeze(2).broadcast_to([128, G, C])
                    nc.vector.tensor_mul(t_cij[:], t_cij[:], rsb)

                def s_pass(t):
                    for cc in range(8):
                        cs = slice(4 * cc, 4 * cc + 4)
                        tmps = tp_pool.tile([128, 4, O, G], F32, tag="tmp")
                        out_ap = tmps[:].transpose([0, 3, 1, 2])
                        in1 = t_cij[:, :, cs].unsqueeze(3) \
                            .broadcast_to([128, G, 4, O])
                        nc.vector.tensor_tensor(out_ap, u4[:, :, cs, :], in1,
                                                mybir.AluOpType.mult)
                        nc.vector.tensor_reduce(t_spart[:, cs, :], tmps[:],
                                                axis=mybir.AxisListType.X,
                                                op=mybir.AluOpType.add)
                    spf = t_spart[:].rearrange("p c o -> p (c o)")
                    for q in range(NQ):
                        pq = sps.tile([B, Q], F32, tag=f"sq{q}", name=f"sq{q}_{t}")
                        nc.tensor.matmul(pq[:], t_d1[:],
                                         spf[:, Q * q:Q * q + Q],
                                         start=True, stop=True)
                        nc.scalar.copy(t_sbounce[:, Q * q:Q * q + Q], pq[:])
                    allreduce(t)

                # iteration 0: s0 already in PSUM (delta/32 matmuls)
                for q in range(NQ):
                    nc.scalar.copy(t_sbounce[:, Q * q:Q * q + Q], s0q[q][:])
                allreduce(0)
                squash(0)
                v_to(d_vdr[0])
                vrep_from(d_vdr[0])
                a_pass(first=True)

                softmax()
                s_pass(1)
                squash(1)
                v_to(d_vdr[1])
                vrep_from(d_vdr[1])
                a_pass(first=False)

                softmax()
                s_pass(2)
                squash(2)
                v_to(d_vout)

    _legalize_install(nc)
    return nc


def _prep_inputs(x, W):
    x_t = np.ascontiguousarray(x.transpose(2, 1, 0))          # [I, R, B]
    xh = x_t.astype(np.float16)
    xl = (x_t - xh.astype(np.float32)).astype(np.float16)
    W_t = np.ascontiguousarray(W.transpose(0, 3, 1, 2)).reshape(R, I, CO)
    Wh = W_t.astype(np.float16)
    Wl = (W_t - Wh.astype(np.float32)).astype(np.float16)
    d0 = np.tile(np.eye(B, dtype=np.float32) / C, (J, 1))
    d1 = np.tile(np.eye(B, dtype=np.float32), (J, 1))
    in_maps = []
    for k in range(NCORES):
        rk = slice(RL * k, RL * (k + 1))
        in_maps.append({
            "xh": np.ascontiguousarray(xh[:, rk, :]),
            "xl": np.ascontiguousarray(xl[:, rk, :]),
            "Wh": Wh[rk], "Wl": Wl[rk],
            "delta_s0": d0, "delta_1": d1,
        })
    return in_maps


def kernel(x: np.ndarray, W: np.ndarray, **run_kwargs) -> np.ndarray:
    if "nc" not in _cache:
        _cache["nc"] = _build()
    nc = _cache["nc"]
    in_maps = _prep_inputs(np.asarray(x), np.asarray(W))
    res = run_bass_kernel_spmd(nc, in_maps, core_ids=list(range(NCORES)),
                               **run_kwargs)
    v = res.results[0]["v_out"].reshape(B, C, O, 1).astype(np.float32)
    if run_kwargs:
        _cache["last_results"] = res
    return v


# revision 4
# speedup vs baseline: 24041.8511x; 24041.8511x over previous
"""CapsNet dynamic-routing layer on 8 Trainium2 NeuronCores.

Strategy
--------
Shard the R=512 routes across 8 cores (64 each); W is read exactly once
machine-wide. Per core:

  u_hat[b, r, c, o] = sum_i W[r,c,o,i] * x[b,r,i]
    via TensorE: stationary = x[r] as [I=128, B=32] fp16 hi/lo pairs,
    moving = W[r] as [I=128, co-chunk 512] fp16 hi/lo pairs, 3 passes
    (hh + lh + hl) accumulated in PSUM -> ~fp32 precision at bf16 speed.
    4 routes run concurrently via tile_position col-strips; PSUM bank
    [128=(rj, b), 512] evacuated to SBUF u_hat [128, g, co].

  Routing iteration 0's s = (1/C) sum_r u_hat comes free on TensorE:
    block-diag delta/32 stationary matmuls accumulate over r in PSUM.

  Iterations 1-2: c_ij mult + segmented reductions on VectorE; the
  cross-partition (rj) sum of s via fp32 delta matmul; softmax/exp/sqrt
  on ScalarE; s AllReduced across cores through DRAM (256 KB).

  Output v = squash(s) computed in a c-packed [128, 512] layout (8x
  cheaper reciprocal), broadcast back through DRAM for the agreement
  passes. All cores produce identical v; core 0's copy is returned.

Numerics: fp16 hi+lo splits carry ~22 mantissa bits; measured end-to-end
error matches pure-fp32 arithmetic (~1.3e-4 on v, routing amplifies any
u_hat error ~1000x, which rules out bf16/tf32 anywhere on the hot path).
"""
import sys

sys.path.insert(0, "/opt/trn_rl_repo")

import numpy as np

import concourse.bass as bass
import concourse.tile as tile
from concourse import mybir
from concourse.bass_utils import run_bass_kernel_spmd

F16 = mybir.dt.float16
F32 = mybir.dt.float32

NCORES = 8
B, R, C, O, I = 32, 512, 32, 64, 128
CO = C * O                # 2048
RL = R // NCORES          # 64 routes per core
J = 4                     # col-strips (rj)
G = RL // J               # 16 r-groups
NQ = 4                    # co chunks
Q = CO // NQ              # 512
C8 = C // 4               # free-c in packed layout
EPS = 1e-8

_cache = {}


def _legalize_install(nc):
    """This walrus build accepts at most one sync wait per instruction and
    none on Matmult; hoist extras onto standalone EventSemaphore ops."""
    import json
    from concourse import mybir as _mb

    def legalize(raw: bytes) -> bytes:
        d = json.loads(raw)
        ctr = 0
        for f in d.get("functions", []):
            for blk in f.get("blocks", []):
                out = []
                for ins in blk.get("instructions", []):
                    si = ins.get("sync_info")
                    waits = (si or {}).get("on_wait") or []
                    keep = 0 if ins.get("opcode") in ("Matmult", "Ldweights") else 1
                    if len(waits) > keep:
                        nh = len(waits) - keep
                        for w in waits[:nh]:
                            ctr += 1
                            out.append({
                                "debug": ins.get("debug", 0),
                                "engine": ins["engine"],
                                "ins": [], "outs": [],
                                "name": f"lgl_wait_{ctr}",
                                "opcode": "EventSemaphore",
                                "sync_info": {"on_update": [], "on_wait": [w]},
                            })
                        si["on_wait"] = waits[nh:]
                    out.append(ins)
                blk["instructions"] = out
        return json.dumps(d).encode()

    nc.to_json_bytes = lambda: legalize(_mb.module_to_json_bytes(nc.m))
    return nc


def _build():
    nc = bass.Bass(trn_type="TRN2", target_bir_lowering=False, debug=False,
                   num_devices=NCORES)

    d_xh = nc.dram_tensor("xh", [I, RL, B], F16, kind="ExternalInput").ap()
    d_xl = nc.dram_tensor("xl", [I, RL, B], F16, kind="ExternalInput").ap()
    d_Wh = nc.dram_tensor("Wh", [RL, I, CO], F16, kind="ExternalInput").ap()
    d_Wl = nc.dram_tensor("Wl", [RL, I, CO], F16, kind="ExternalInput").ap()
    d_d0 = nc.dram_tensor("delta_s0", [128, B], F32, kind="ExternalInput").ap()
    d_d1 = nc.dram_tensor("delta_1", [128, B], F32, kind="ExternalInput").ap()
    d_vout = nc.dram_tensor("v_out", [B, CO], F32, kind="ExternalOutput").ap()

    d_sb = [nc.dram_tensor(f"s_bounce{t}", [B, CO], F32).ap() for t in range(3)]
    d_sr = [nc.dram_tensor(f"s_red{t}", [B, CO], F32, addr_space="Shared").ap()
            for t in range(3)]
    d_vdr = [nc.dram_tensor(f"v_dr{t}", [B, CO], F32).ap() for t in range(2)]

    groups = [list(range(NCORES))]

    with tile.TileContext(nc) as tc:
        with tc.tile_pool(name="const", bufs=1) as cpool, \
             tc.tile_pool(name="upool", bufs=1) as upool, \
             tc.tile_pool(name="s0ps", bufs=1, space="PSUM") as s0ps:

            t_xh = cpool.tile([I, RL * B], F16, tag="xh")
            t_xl = cpool.tile([I, RL * B], F16, tag="xl")
            nc.sync.dma_start(t_xh[:].rearrange("i (r b) -> i r b", r=RL), d_xh)
            nc.sync.dma_start(t_xl[:].rearrange("i (r b) -> i r b", r=RL), d_xl)
            t_d0 = cpool.tile([128, B], F32, tag="d0")
            t_d1 = cpool.tile([128, B], F32, tag="d1")
            nc.sync.dma_start(t_d0[:], d_d0)
            nc.sync.dma_start(t_d1[:], d_d1)
            t_eps = cpool.tile([128, 1], F32, tag="eps")
            nc.gpsimd.memset(t_eps[:], EPS)

            t_u = upool.tile([128, G, CO], F32, tag="u")
            t_acc = upool.tile([128, NQ, Q], F32, tag="s0acc")
            s0q = [s0ps.tile([B, Q], F32, tag=f"s0q{q}", name=f"s0q{q}")
                   for q in range(NQ)]

            # ---- Phase A: u_hat production + iter-0 s accumulation ----
            # s0 rides the evacuation path on DVE/ACT (which idle under the
            # DMA floor) instead of costing TensorE fp32 matmul cycles.
            with tc.tile_pool(name="wpool", bufs=5) as wpool, \
                 tc.tile_pool(name="prodps", bufs=3, space="PSUM") as prodps:
                for g in range(G):
                    for q in range(NQ):
                        wh = wpool.tile([I, J, Q], F16, tag="wh")
                        wl = wpool.tile([I, J, Q], F16, tag="wl")
                        nc.sync.dma_start(
                            wh[:], d_Wh[J * g:J * g + J, :, Q * q:Q * q + Q]
                            .transpose([1, 0, 2]))
                        nc.sync.dma_start(
                            wl[:], d_Wl[J * g:J * g + J, :, Q * q:Q * q + Q]
                            .transpose([1, 0, 2]))
                        pp = prodps.tile([128, Q], F32, tag="prod")
                        for j in range(J):
                            r = J * g + j
                            sxh = t_xh[:, r * B:(r + 1) * B]
                            sxl = t_xl[:, r * B:(r + 1) * B]
                            tp = (0, 32 * j)
                            ppj = pp[32 * j:32 * (j + 1), :]
                            nc.tensor.matmul(ppj, sxh, wh[:, j, :],
                                             start=True, stop=False,
                                             tile_position=tp)
                            nc.tensor.matmul(ppj, sxl, wh[:, j, :],
                                             start=False, stop=False,
                                             tile_position=tp)
                            nc.tensor.matmul(ppj, sxh, wl[:, j, :],
                                             start=False, stop=True,
                                             tile_position=tp)
                        useg = t_u[:, g, Q * q:Q * q + Q]
                        accq = t_acc[:, q, :]
                        # u-evac copies alternate DVE/ACT; the s0 accumulate
                        # is tensor+tensor so it must stay on DVE.
                        if (g + q) % 2 == 0:
                            nc.vector.tensor_copy(useg, pp[:])
                        else:
                            nc.scalar.copy(useg, pp[:])
                        if g == 0:
                            nc.scalar.copy(accq, pp[:])
                        else:
                            nc.vector.tensor_add(accq, accq, pp[:])
                # rj-sum of the accumulator (tiny fp32 delta matmuls; the
                # 1/C of iteration 0's uniform c_ij is baked into t_d0)
                for q in range(NQ):
                    nc.tensor.matmul(s0q[q][:], t_d0[:], t_acc[:, q, :],
                                     start=True, stop=True)

            # ---- Phase B: routing iterations ----
            with tc.tile_pool(name="iter", bufs=1) as ip, \
                 tc.tile_pool(name="tmp", bufs=1) as tp_pool, \
                 tc.tile_pool(name="sps", bufs=1, space="PSUM") as sps:

                t_vrep = ip.tile([128, CO], F32, tag="vrep")
                t_spart = ip.tile([128, C, O], F32, tag="spart")
                t_b = ip.tile([128, G, C], F32, tag="bij")
                t_a = ip.tile([128, G, C], F32, tag="aij")
                t_cij = ip.tile([128, G, C], F32, tag="cij")
                t_mx = ip.tile([128, G], F32, tag="mx")
                t_rs = ip.tile([128, G], F32, tag="rs")
                t_sbounce = ip.tile([B, CO], F32, tag="sbounce")
                t_spk = ip.tile([128, NQ * C8 * O // NQ], F32, tag="spk")
                t_sq = ip.tile([128, C8 * O], F32, tag="sqt")
                t_rt = ip.tile([128, C8 * O], F32, tag="rt")
                t_vpk = ip.tile([128, C8 * O], F32, tag="vpk")

                u4 = t_u[:].rearrange("p g (c o) -> p g c o", c=C)

                def allreduce(t):
                    nc.sync.dma_start(d_sb[t], t_sbounce[:])
                    nc.gpsimd.collective_compute(
                        "AllReduce", mybir.AluOpType.add,
                        replica_groups=groups,
                        ins=[d_sb[t].opt()], outs=[d_sr[t].opt()])

                def squash(t):
                    """d_sr[t] -> packed v in t_vpk; partitions (c4, b),
                    free (c8, o) with c = c8*4 + c4."""
                    sr4 = d_sr[t].rearrange("b (c8 c4 o) -> b c8 c4 o",
                                            c4=4, o=O)
                    for c4 in range(4):
                        nc.sync.dma_start(
                            t_spk[32 * c4:32 * (c4 + 1), :]
                            .rearrange("b (c8 o) -> b c8 o", c8=C8),
                            sr4[:, :, c4, :])
                    nc.scalar.square(t_sq[:], t_spk[:])
                    nc.scalar.activation(t_rt[:], t_sq[:],
                                         mybir.ActivationFunctionType.Sqrt,
                                         bias=t_eps[:])
                    # rt := (1+sq)*sqrt(sq+eps); vpk := sq*s; rt := 1/rt
                    nc.vector.tensor_scalar_add(t_vpk[:], t_sq[:], 1.0)
                    nc.vector.tensor_mul(t_rt[:], t_rt[:], t_vpk[:])
                    nc.vector.reciprocal(t_rt[:], t_rt[:])
                    nc.vector.tensor_mul(t_sq[:], t_sq[:], t_spk[:])
                    nc.vector.tensor_mul(t_vpk[:], t_sq[:], t_rt[:])

                def v_to(dst):
                    d4 = dst.rearrange("b (c8 c4 o) -> b c8 c4 o", c4=4, o=O)
                    for c4 in range(4):
                        nc.sync.dma_start(
                            d4[:, :, c4, :],
                            t_vpk[32 * c4:32 * (c4 + 1), :]
                            .rearrange("b (c8 o) -> b c8 o", c8=C8))

                def vrep_from(src):
                    sflat = src.rearrange("b co -> b co")
                    for j in range(J):
                        nc.sync.dma_start(t_vrep[32 * j:32 * (j + 1), :], sflat)

                def a_pass(first):
                    dst = t_b if first else t_a
                    vr4 = t_vrep[:].rearrange("p (c o) -> p c o", c=C)
                    for cc in range(8):
                        cs = slice(4 * cc, 4 * cc + 4)
                        tmp2 = tp_pool.tile([128, G, 4, O], F32, tag="tmp")
                        in1 = vr4[:, cs, :].unsqueeze(1) \
                            .broadcast_to([128, G, 4, O])
                        nc.vector.tensor_tensor(tmp2[:], u4[:, :, cs, :], in1,
                                                mybir.AluOpType.mult)
                        nc.vector.tensor_reduce(dst[:, :, cs], tmp2[:],
                                                axis=mybir.AxisListType.X,
                                                op=mybir.AluOpType.add)
                    if not first:
                        nc.vector.tensor_add(t_b[:], t_b[:], t_a[:])

                def softmax():
                    nc.vector.tensor_reduce(t_mx[:], t_b[:],
                                            axis=mybir.AxisListType.X,
                                            op=mybir.AluOpType.max)
                    mxb = t_mx[:].unsqueeze(2).broadcast_to([128, G, C])
                    nc.vector.tensor_sub(t_cij[:], t_b[:], mxb)
                    nc.scalar.activation(t_cij[:], t_cij[:],
                                         mybir.ActivationFunctionType.Exp)
                    nc.vector.tensor_reduce(t_rs[:], t_cij[:],
                                            axis=mybir.AxisListType.X,
                                            op=mybir.AluOpType.add)
                    nc.vector.reciprocal(t_rs[:], t_rs[:])
                    rsb = t_rs[:].unsqueeze(2).broadcast_to([128, G, C])
                    nc.vector.tensor_mul(t_cij[:], t_cij[:], rsb)

                def s_pass(t):
                    for cc in range(8):
                        cs = slice(4 * cc, 4 * cc + 4)
                        tmps = tp_pool.tile([128, 4, O, G], F32, tag="tmp")
                        out_ap = tmps[:].transpose([0, 3, 1, 2])
                        in1 = t_cij[:, :, cs].unsqueeze(3) \
                            .broadcast_to([128, G, 4, O])
                        nc.vector.tensor_tensor(out_ap, u4[:, :, cs, :], in1,
                                                mybir.AluOpType.mult)
                        nc.vector.tensor_reduce(t_spart[:, cs, :], tmps[:],
                                                axis=mybir.AxisListType.X,
                                                op=mybir.AluOpType.add)
                    spf = t_spart[:].rearrange("p c o -> p (c o)")
                    for q in range(NQ):
                        pq = sps.tile([B, Q], F32, tag=f"sq{q}", name=f"sq{q}_{t}")
                        nc.tensor.matmul(pq[:], t_d1[:],
                                         spf[:, Q * q:Q * q + Q],
                                         start=True, stop=True)
                        nc.scalar.copy(t_sbounce[:, Q * q:Q * q + Q], pq[:])
                    allreduce(t)

                # iteration 0: s0 already in PSUM (delta/32 matmuls)
                for q in range(NQ):
                    nc.scalar.copy(t_sbounce[:, Q * q:Q * q + Q], s0q[q][:])
                allreduce(0)
                squash(0)
                v_to(d_vdr[0])
                vrep_from(d_vdr[0])
                a_pass(first=True)

                softmax()
                s_pass(1)
                squash(1)
                v_to(d_vdr[1])
                vrep_from(d_vdr[1])
                a_pass(first=False)

                softmax()
                s_pass(2)
                squash(2)
                v_to(d_vout)

    _legalize_install(nc)
    return nc


def _prep_inputs(x, W):
    x_t = np.ascontiguousarray(x.transpose(2, 1, 0))          # [I, R, B]
    xh = x_t.astype(np.float16)
    xl = (x_t - xh.astype(np.float32)).astype(np.float16)
    W_t = np.ascontiguousarray(W.transpose(0, 3, 1, 2)).reshape(R, I, CO)
    Wh = W_t.astype(np.float16)
    Wl = (W_t - Wh.astype(np.float32)).astype(np.float16)
    d0 = np.tile(np.eye(B, dtype=np.float32) / C, (J, 1))
    d1 = np.tile(np.eye(B, dtype=np.float32), (J, 1))
    in_maps = []
    for k in range(NCORES):
        rk = slice(RL * k, RL * (k + 1))
        in_maps.append({
            "xh": np.ascontiguousarray(xh[:, rk, :]),
            "xl": np.ascontiguousarray(xl[:, rk, :]),
            "Wh": Wh[rk], "Wl": Wl[rk],
            "delta_s0": d0, "delta_1": d1,
        })
    return in_maps


def kernel(x: np.ndarray, W: np.ndarray, **run_kwargs) -> np.ndarray:
    if "nc" not in _cache:
        _cache["nc"] = _build()
    nc = _cache["nc"]
    in_maps = _prep_inputs(np.asarray(x), np.asarray(W))
    res = run_bass_kernel_spmd(nc, in_maps, core_ids=list(range(NCORES)),
                               **run_kwargs)
    v = res.results[0]["v_out"].reshape(B, C, O, 1).astype(np.float32)
    if run_kwargs:
        _cache["last_results"] = res
    return v
